# revision 32
# baseline (speedup 1.0000x reference)
"""ClusterLoss (vq codebook coverage entropy) Trainium2 kernel.

Problem (hardcoded shapes):
    selected_frames [B=512, K=64, D=512] f32, cluster_centers [N=1024, D=512] f32.
    assignments = argmin_n ||x_{b,k} - c_n||^2
    coverage[n]  = sum_b 1[any_k assignments[b,k] == n]
    prob = coverage / (B*K);  out = -sum prob*log(prob+1e-10)   (scalar f32)

Strategy:
    * Data-parallel over batch: 64 batch elements (4096 frames) per core on 8
      cores; cluster_centers replicated.  Frames are pre-transposed on the
      host so every matmul operand is D-major, all in bf16 (measured effect:
      ~127/32768 flipped assignments, ~6.5e-4 relative entropy error).
    * argmin_n dist^2 == argmax_n s,  s = cross - 0.5*||c_n||^2.  Per
      128-frame tile, cross accumulates into fp32 PSUM via 8 bf16 matmuls at
      peak rate.
    * variant "packed" (default): the -0.5||c||^2 bias is added in-PSUM by a
      rank-2 bf16 (hi/lo) matmul pair; the two n-halves are placed in PE
      row-groups 0 and 1 (operands at base partitions 0 and 32) so they run
      concurrently in the array.  VectorE then does one PSUM row-max;
      ScalarE computes mask = sign(m - s) in {0 (argmax), 1}.
    * count[b, n] = sum_k mask via a block-ones stationary matmul pair,
      accumulated over all 32 tiles in persistent PSUM; the two n-halves go
      to PE column-groups (out partitions 0-63 / 64-127) and also run
      concurrently.
    * count PSUM -> SBUF -> DRAM; host: coverage[n] = sum_b 1[count<=63.5],
      then prob/entropy over the 8 per-core blocks.
    * variant "nottr": bias added on VectorE (tensor_tensor) instead of the
      rank-2 matmuls; row-max from SBUF.  Kept for A/B comparison.
"""

import os
import numpy as np
import ml_dtypes

import concourse.bass as bass
import concourse.mybir as mybir
import concourse.tile as tile
from concourse import bacc
from concourse.bass_utils import run_bass_kernel_spmd

BF16 = ml_dtypes.bfloat16
VARIANT = os.environ.get("KERNEL_VARIANT", "nottrpc")

B, K, D, N = 512, 64, 512, 1024
NCORES = 8
B_PER_CORE = B // NCORES          # 64
F_PER_CORE = B_PER_CORE * K       # 4096 frames
NT = F_PER_CORE // 128            # 32 tiles of 128 frames (2 batch elems)
GROUP = 8                         # f-tiles per frame-load group
SUB = 2                           # DMAs per group
COUNT_DELAY = 3                   # tiles to delay the count matmul (pipelining)

_CACHE = {}
LAST_RESULTS = None
RUN_KWARGS = {}  # extra kwargs for run_bass_kernel_spmd (profiling harness hook)


def _build_nc():
    nc = bacc.Bacc("TRN2", target_bir_lowering=False, debug=False,
                   num_devices=NCORES)
    dt = mybir.dt
    packed = VARIANT == "packed"

    # DRAM I/O (per core).  fT layout: fT[d, t*512 + c*128 + f] =
    # frames[128*t + f, 128*c + d].
    fT = nc.dram_tensor("fT", [128, NT * 512], dt.bfloat16, kind="ExternalInput")
    cT = nc.dram_tensor("cT", [4, 128, N], dt.bfloat16, kind="ExternalInput")
    bmap = nc.dram_tensor("bmap", [128, 127], dt.bfloat16, kind="ExternalInput")
    cnt_out = nc.dram_tensor("cnt", [128, N], dt.float32, kind="ExternalOutput")
    if packed:
        csq2 = nc.dram_tensor("csq2", [2, N], dt.bfloat16, kind="ExternalInput")
    else:
        csqb = nc.dram_tensor("csqb", [128, N], dt.float32, kind="ExternalInput")

    with tile.TileContext(nc) as tc:
        with (
            tc.tile_pool(name="const", bufs=1) as cpool,
            tc.tile_pool(name="frames", bufs=2) as fpool,
            tc.tile_pool(name="sbias", bufs=3) as sbpool,
            tc.tile_pool(name="mask", bufs=COUNT_DELAY + 2) as mpool,
            tc.tile_pool(name="mrow", bufs=4) as mnpool,
            tc.tile_pool(name="scratch", bufs=2) as scpool,
            tc.tile_pool(name="spsum", bufs=2, space="PSUM") as spool,
            tc.tile_pool(name="cpsum", bufs=1, space="PSUM") as kpool,
            tc.tile_pool(name="outp", bufs=1) as opool,
        ):
            SUBW = GROUP * 512 // SUB
            fgs = {}

            def load_group(g):
                for u in range(SUB):
                    fg = fpool.tile([128, SUBW], dt.bfloat16, tag=f"fg{u}",
                                    name=f"fg{g}_{u}")
                    fgs[(g, u)] = fg
                    nc.gpsimd.dma_start(
                        out=fg[:],
                        in_=fT[:, g * GROUP * 512 + u * SUBW:
                               g * GROUP * 512 + (u + 1) * SUBW])

            load_group(0)

            ct = []
            for c in range(4):
                t_ = cpool.tile([128, N], dt.bfloat16, tag=f"ct{c}", name=f"ctt{c}")
                nc.sync.dma_start(out=t_[:], in_=cT[c])
                ct.append(t_)
            bmap_t = cpool.tile([128, 127], dt.bfloat16, tag="bmap", name="bmap_t")
            nc.sync.dma_start(out=bmap_t[:], in_=bmap[:])

            if packed:
                # bias operands at base partitions 0 (h=0) and 32 (h=1):
                # auto-derived tile_position puts the two rank-2 bias matmuls
                # in different PE row-groups so they run concurrently.
                csq2_t = cpool.tile([34, N], dt.bfloat16, tag="csq2",
                                    name="csq2_t")
                nc.sync.dma_start(out=csq2_t[0:2, :], in_=csq2[:])
                nc.sync.dma_start(out=csq2_t[32:34, :], in_=csq2[:])
                ones2_t = cpool.tile([34, 128], dt.bfloat16, tag="ones2",
                                     name="ones2_t")
                nc.vector.memset(ones2_t[0:2, :], 1.0)
                nc.vector.memset(ones2_t[32:34, :], 1.0)
                bias_ops = [(ones2_t[0:2, :], csq2_t[0:2, :]),
                            (ones2_t[32:34, :], csq2_t[32:34, :])]
            else:
                csqb_t = cpool.tile([128, N], dt.float32, tag="csqb",
                                    name="csqb_t")
                nc.sync.dma_start(out=csqb_t[:], in_=csqb[:])

            # persistent count accumulator: count = 64 - #argmax-hits.
            # Quadrant layout [128, 1024]: h=0 in rows 0:64 cols 0:512
            # (bank 0), h=1 in rows 64:128 cols 512:1024 (bank 1) — the two
            # count matmuls land in different PE column-groups and run
            # concurrently in the array.
            count = kpool.tile([128, N], dt.float32, tag="count", name="count")

            pending = []  # (t, mask_tile) with deferred count-matmul

            def flush_one():
                t, m = pending.pop(0)
                for h in range(2):
                    nc.tensor.matmul(
                        count[64 * h:64 * h + 64, h * 512:(h + 1) * 512],
                        lhsT=bmap_t[:, 63 - 2 * t:127 - 2 * t],
                        rhs=m[:, h * 512:(h + 1) * 512],
                        start=(t == 0), stop=(t == NT - 1),
                    )

            for g in range(NT // GROUP):
                if g + 1 < NT // GROUP:
                    load_group(g + 1)  # double-buffered prefetch
                for j in range(GROUP):
                    t = g * GROUP + j
                    fg = fgs[(g, j // (GROUP // SUB))]
                    jj = j % (GROUP // SUB)
                    s = spool.tile([128, N], dt.float32, tag="s", name=f"s{t}")
                    last_chunk = -1 if packed else 3
                    for c in range(4):
                        lhsT = fg[:, jj * 512 + c * 128: jj * 512 + (c + 1) * 128]
                        for h in range(2):
                            nc.tensor.matmul(
                                s[:, h * 512:(h + 1) * 512],
                                lhsT=lhsT,
                                rhs=ct[c][:, h * 512:(h + 1) * 512],
                                start=(c == 0), stop=(c == last_chunk),
                            )
                    mrow = mnpool.tile([128, 1], dt.float32, tag="mrow",
                                       name=f"mrow{t}")
                    if packed:
                        for h in range(2):
                            o2, cs2 = bias_ops[h]
                            nc.tensor.matmul(
                                s[:, h * 512:(h + 1) * 512],
                                lhsT=o2,
                                rhs=cs2[:, h * 512:(h + 1) * 512],
                                start=False, stop=True,
                            )
                        cmp_src = s
                        nc.vector.reduce_max(out=mrow[:], in_=s[:],
                                             axis=mybir.AxisListType.X)
                    else:
                        sb = sbpool.tile([128, N], dt.float32, tag="sb",
                                         name=f"sb{t}")
                        nc.vector.tensor_tensor(
                            out=sb[:], in0=s[:], in1=csqb_t[:],
                            op=mybir.AluOpType.add)
                        if VARIANT == "nottrts":
                            # row-max via single-src tensor_scalar accumulate:
                            # SBUF fp32 single-src runs in the DVE 2x port
                            # mode, ~2x faster than tensor_reduce.
                            dmy = scpool.tile([128, N], dt.bfloat16,
                                              tag="dmy", name=f"dmy{t}")
                            nc.vector.tensor_scalar(
                                dmy[:], sb[:], 0.0, None,
                                mybir.AluOpType.add, mybir.AluOpType.max,
                                accum_out=mrow[:])
                        else:
                            nc.vector.reduce_max(out=mrow[:], in_=sb[:],
                                                 axis=mybir.AxisListType.X)
                        cmp_src = sb
                    # mask = sign(m - s) in {0 (argmax), 1 (other)}
                    mask = mpool.tile([128, N], dt.bfloat16, tag="mask",
                                      name=f"mask{t}")
                    nc.scalar.activation(mask[:], cmp_src[:],
                                         mybir.ActivationFunctionType.Sign,
                                         bias=mrow[:], scale=-1.0)
                    pending.append((t, mask))
                    if len(pending) > COUNT_DELAY:
                        flush_one()
            while pending:
                flush_one()

            res = opool.tile([128, N], dt.float32, tag="res", name="res")
            for h in range(2):
                sl = (slice(64 * h, 64 * h + 64), slice(h * 512, (h + 1) * 512))
                nc.scalar.copy(out=res[sl], in_=count[sl])
                nc.sync.dma_start(out=cnt_out[sl], in_=res[sl])

    nc.compile()
    return nc


def _get_nc():
    if "nc" not in _CACHE:
        _CACHE["nc"] = _build_nc()
    return _CACHE["nc"]


def _prep_inputs(selected_frames: np.ndarray, cluster_centers: np.ndarray):
    frames = np.ascontiguousarray(np.asarray(selected_frames, dtype=np.float32))
    centers = np.ascontiguousarray(np.asarray(cluster_centers, dtype=np.float32))

    # centers^T, D-major, chunked into 4 partition blocks of 128.
    cT = np.ascontiguousarray(centers.T.reshape(4, 128, N).astype(BF16))

    # -0.5*||c||^2: fp32 replicated (nottr) and bf16 hi/lo (packed).
    v = (-0.5 * (centers.astype(np.float64) ** 2).sum(-1)).astype(np.float32)
    csqb = np.ascontiguousarray(np.broadcast_to(v, (128, N)))
    hi = v.astype(BF16)
    lo = (v - hi.astype(np.float32)).astype(BF16)
    csq2 = np.ascontiguousarray(np.stack([hi, lo]))

    # block-ones map: column window [63-2t, 127-2t) is the [128, 64]
    # stationary matrix sending frame row f to batch row 2t + f//64.
    bmap = np.zeros((128, 127), dtype=BF16)
    bmap[0:64, 63] = 1
    bmap[64:128, 64] = 1

    in_maps = []
    fl = frames.reshape(B, K, D)
    for core in range(NCORES):
        fc = fl[core * B_PER_CORE:(core + 1) * B_PER_CORE].reshape(F_PER_CORE, D)
        fTc = np.ascontiguousarray(
            fc.reshape(NT, 128, 4, 128).transpose(3, 0, 2, 1)
        ).reshape(128, NT * 512).astype(BF16)
        fTc = np.ascontiguousarray(fTc)
        in_maps.append({"fT": fTc, "cT": cT, "csqb": csqb, "csq2": csq2,
                        "bmap": bmap})
    return in_maps


def _counts_to_cov(cnt: np.ndarray) -> np.ndarray:
    """Per-core count block -> coverage histogram [N]."""
    if cnt.shape[0] == 128:  # packed quadrant layout
        c0 = cnt[0:64, 0:512]
        c1 = cnt[64:128, 512:1024]
        cnt = np.concatenate([c0, c1], axis=1)
    return (cnt <= 63.5).sum(axis=0)


def kernel(selected_frames: np.ndarray, cluster_centers: np.ndarray) -> np.ndarray:
    global LAST_RESULTS
    nc = _get_nc()
    in_maps = _prep_inputs(selected_frames, cluster_centers)
    res = run_bass_kernel_spmd(nc, in_maps, list(range(NCORES)), **RUN_KWARGS)
    LAST_RESULTS = res
    cov = np.zeros(N, dtype=np.float64)
    for core in range(NCORES):
        cov += _counts_to_cov(res.results[core]["cnt"])
    prob = cov / (B * K)
    entropy = -(prob * np.log(prob + 1e-10)).sum()
    return np.float32(entropy)


if __name__ == "__main__":
    rng = np.random.default_rng(0)
    sf = rng.standard_normal((B, K, D), dtype=np.float32)
    cc = rng.standard_normal((N, D), dtype=np.float32)
    out = kernel(sf, cc)
    print("kernel out:", out)


# revision 34
# speedup vs baseline: 1.1918x; 1.1918x over previous
"""ClusterLoss (vq codebook coverage entropy) Trainium2 kernel.

Problem (hardcoded shapes):
    selected_frames [B=512, K=64, D=512] f32, cluster_centers [N=1024, D=512] f32.
    assignments = argmin_n ||x_{b,k} - c_n||^2
    coverage[n]  = sum_b 1[any_k assignments[b,k] == n]
    prob = coverage / (B*K);  out = -sum prob*log(prob+1e-10)   (scalar f32)

Strategy:
    * Data-parallel over batch: 64 batch elements (4096 frames) per core on 8
      cores; cluster_centers replicated.  Frames are pre-transposed on the
      host so every matmul operand is D-major, all in bf16 (measured effect:
      ~127/32768 flipped assignments, ~6.5e-4 relative entropy error).
    * argmin_n dist^2 == argmax_n s,  s = cross - 0.5*||c_n||^2.  Per
      128-frame tile, cross accumulates into fp32 PSUM via 8 bf16 matmuls at
      peak rate.
    * variant "packed" (default): the -0.5||c||^2 bias is added in-PSUM by a
      rank-2 bf16 (hi/lo) matmul pair; the two n-halves are placed in PE
      row-groups 0 and 1 (operands at base partitions 0 and 32) so they run
      concurrently in the array.  VectorE then does one PSUM row-max;
      ScalarE computes mask = sign(m - s) in {0 (argmax), 1}.
    * count[b, n] = sum_k mask via a block-ones stationary matmul pair,
      accumulated over all 32 tiles in persistent PSUM; the two n-halves go
      to PE column-groups (out partitions 0-63 / 64-127) and also run
      concurrently.
    * count PSUM -> SBUF -> DRAM; host: coverage[n] = sum_b 1[count<=63.5],
      then prob/entropy over the 8 per-core blocks.
    * variant "nottr": bias added on VectorE (tensor_tensor) instead of the
      rank-2 matmuls; row-max from SBUF.  Kept for A/B comparison.
"""

import os
import numpy as np
import ml_dtypes

import concourse.bass as bass
import concourse.mybir as mybir
import concourse.tile as tile
from concourse import bacc
from concourse.bass_utils import run_bass_kernel_spmd

BF16 = ml_dtypes.bfloat16
VARIANT = os.environ.get("KERNEL_VARIANT", "nottrpc")

B, K, D, N = 512, 64, 512, 1024
NCORES = 8
B_PER_CORE = B // NCORES          # 64
F_PER_CORE = B_PER_CORE * K       # 4096 frames
NT = F_PER_CORE // 128            # 32 tiles of 128 frames (2 batch elems)
GROUP = 8                         # f-tiles per frame-load group
SUB = 2                           # DMAs per group
COUNT_DELAY = 3                   # tiles to delay the count matmul (pipelining)

_CACHE = {}
LAST_RESULTS = None
RUN_KWARGS = {}  # extra kwargs for run_bass_kernel_spmd (profiling harness hook)


def _build_nc():
    nc = bacc.Bacc("TRN2", target_bir_lowering=False, debug=False,
                   num_devices=NCORES)
    dt = mybir.dt
    packed = VARIANT == "packed"

    # DRAM I/O (per core).  fT layout: fT[d, t*512 + c*128 + f] =
    # frames[128*t + f, 128*c + d].
    fT = nc.dram_tensor("fT", [128, NT * 512], dt.bfloat16, kind="ExternalInput")
    cT = nc.dram_tensor("cT", [4, 128, N], dt.bfloat16, kind="ExternalInput")
    bmap = nc.dram_tensor("bmap", [128, 127], dt.bfloat16, kind="ExternalInput")
    cnt_out = nc.dram_tensor("cnt", [128, N], dt.float32, kind="ExternalOutput")
    if packed:
        csq2 = nc.dram_tensor("csq2", [2, N], dt.bfloat16, kind="ExternalInput")
    else:
        csqb = nc.dram_tensor("csqb", [128, N], dt.float32, kind="ExternalInput")

    with tile.TileContext(nc) as tc:
        with (
            tc.tile_pool(name="const", bufs=1) as cpool,
            tc.tile_pool(name="frames", bufs=2) as fpool,
            tc.tile_pool(name="sbias", bufs=3) as sbpool,
            tc.tile_pool(name="mask", bufs=COUNT_DELAY + 2) as mpool,
            tc.tile_pool(name="mrow", bufs=4) as mnpool,
            tc.tile_pool(name="scratch", bufs=2) as scpool,
            tc.tile_pool(name="spsum", bufs=2, space="PSUM") as spool,
            tc.tile_pool(name="cpsum", bufs=1, space="PSUM") as kpool,
            tc.tile_pool(name="outp", bufs=1) as opool,
        ):
            SUBW = GROUP * 512 // SUB
            fgs = {}

            def load_group(g):
                for u in range(SUB):
                    fg = fpool.tile([128, SUBW], dt.bfloat16, tag=f"fg{u}",
                                    name=f"fg{g}_{u}")
                    fgs[(g, u)] = fg
                    nc.gpsimd.dma_start(
                        out=fg[:],
                        in_=fT[:, g * GROUP * 512 + u * SUBW:
                               g * GROUP * 512 + (u + 1) * SUBW])

            # group 0 first half on SWDGE, then csqb (needed by the first ADD)
            # on the same queue, then the rest; centers stream on HWDGE.
            fg00 = fpool.tile([128, SUBW], dt.bfloat16, tag="fg0", name="fg0_0")
            fgs[(0, 0)] = fg00
            nc.gpsimd.dma_start(out=fg00[:], in_=fT[:, 0:SUBW])
            if not packed:
                csqb_t = cpool.tile([128, N], dt.float32, tag="csqb",
                                    name="csqb_t")
                nc.gpsimd.dma_start(out=csqb_t[:], in_=csqb[:])
            fg01 = fpool.tile([128, SUBW], dt.bfloat16, tag="fg1", name="fg0_1")
            fgs[(0, 1)] = fg01
            nc.gpsimd.dma_start(out=fg01[:], in_=fT[:, SUBW:2 * SUBW])

            ct = []
            for c in range(4):
                t_ = cpool.tile([128, N], dt.bfloat16, tag=f"ct{c}", name=f"ctt{c}")
                nc.sync.dma_start(out=t_[:], in_=cT[c])
                ct.append(t_)
            bmap_t = cpool.tile([128, 127], dt.bfloat16, tag="bmap", name="bmap_t")
            nc.sync.dma_start(out=bmap_t[:], in_=bmap[:])

            if packed:
                # bias operands at base partitions 0 (h=0) and 32 (h=1):
                # auto-derived tile_position puts the two rank-2 bias matmuls
                # in different PE row-groups so they run concurrently.
                csq2_t = cpool.tile([34, N], dt.bfloat16, tag="csq2",
                                    name="csq2_t")
                nc.sync.dma_start(out=csq2_t[0:2, :], in_=csq2[:])
                nc.sync.dma_start(out=csq2_t[32:34, :], in_=csq2[:])
                ones2_t = cpool.tile([34, 128], dt.bfloat16, tag="ones2",
                                     name="ones2_t")
                nc.vector.memset(ones2_t[0:2, :], 1.0)
                nc.vector.memset(ones2_t[32:34, :], 1.0)
                bias_ops = [(ones2_t[0:2, :], csq2_t[0:2, :]),
                            (ones2_t[32:34, :], csq2_t[32:34, :])]
            # persistent count accumulator: count = 64 - #argmax-hits.
            # Quadrant layout [128, 1024]: h=0 in rows 0:64 cols 0:512
            # (bank 0), h=1 in rows 64:128 cols 512:1024 (bank 1) — the two
            # count matmuls land in different PE column-groups and run
            # concurrently in the array.
            count = kpool.tile([128, N], dt.float32, tag="count", name="count")

            pending = []  # (t, mask_tile) with deferred count-matmul

            def flush_one():
                t, m = pending.pop(0)
                for h in range(2):
                    nc.tensor.matmul(
                        count[64 * h:64 * h + 64, h * 512:(h + 1) * 512],
                        lhsT=bmap_t[:, 63 - 2 * t:127 - 2 * t],
                        rhs=m[:, h * 512:(h + 1) * 512],
                        start=(t == 0), stop=(t == NT - 1),
                    )

            for g in range(NT // GROUP):
                if g + 1 < NT // GROUP:
                    load_group(g + 1)  # double-buffered prefetch
                for j in range(GROUP):
                    t = g * GROUP + j
                    fg = fgs[(g, j // (GROUP // SUB))]
                    jj = j % (GROUP // SUB)
                    s = spool.tile([128, N], dt.float32, tag="s", name=f"s{t}")
                    last_chunk = -1 if packed else 3
                    for c in range(4):
                        lhsT = fg[:, jj * 512 + c * 128: jj * 512 + (c + 1) * 128]
                        for h in range(2):
                            nc.tensor.matmul(
                                s[:, h * 512:(h + 1) * 512],
                                lhsT=lhsT,
                                rhs=ct[c][:, h * 512:(h + 1) * 512],
                                start=(c == 0), stop=(c == last_chunk),
                            )
                    mrow = mnpool.tile([128, 1], dt.float32, tag="mrow",
                                       name=f"mrow{t}")
                    if packed:
                        for h in range(2):
                            o2, cs2 = bias_ops[h]
                            nc.tensor.matmul(
                                s[:, h * 512:(h + 1) * 512],
                                lhsT=o2,
                                rhs=cs2[:, h * 512:(h + 1) * 512],
                                start=False, stop=True,
                            )
                        cmp_src = s
                        nc.vector.reduce_max(out=mrow[:], in_=s[:],
                                             axis=mybir.AxisListType.X)
                    else:
                        sb = sbpool.tile([128, N], dt.float32, tag="sb",
                                         name=f"sb{t}")
                        nc.vector.tensor_tensor(
                            out=sb[:], in0=s[:], in1=csqb_t[:],
                            op=mybir.AluOpType.add)
                        if VARIANT == "nottrts":
                            # row-max via single-src tensor_scalar accumulate:
                            # SBUF fp32 single-src runs in the DVE 2x port
                            # mode, ~2x faster than tensor_reduce.
                            dmy = scpool.tile([128, N], dt.bfloat16,
                                              tag="dmy", name=f"dmy{t}")
                            nc.vector.tensor_scalar(
                                dmy[:], sb[:], 0.0, None,
                                mybir.AluOpType.add, mybir.AluOpType.max,
                                accum_out=mrow[:])
                        else:
                            nc.vector.reduce_max(out=mrow[:], in_=sb[:],
                                                 axis=mybir.AxisListType.X)
                        cmp_src = sb
                    # mask = sign(m - s) in {0 (argmax), 1 (other)}
                    mask = mpool.tile([128, N], dt.bfloat16, tag="mask",
                                      name=f"mask{t}")
                    nc.scalar.activation(mask[:], cmp_src[:],
                                         mybir.ActivationFunctionType.Sign,
                                         bias=mrow[:], scale=-1.0)
                    pending.append((t, mask))
                    if len(pending) > COUNT_DELAY:
                        flush_one()
            while pending:
                flush_one()

            res = opool.tile([128, N], dt.float32, tag="res", name="res")
            for h in range(2):
                sl = (slice(64 * h, 64 * h + 64), slice(h * 512, (h + 1) * 512))
                nc.scalar.copy(out=res[sl], in_=count[sl])
                nc.sync.dma_start(out=cnt_out[sl], in_=res[sl])

    nc.compile()
    return nc


def _get_nc():
    if "nc" not in _CACHE:
        _CACHE["nc"] = _build_nc()
    return _CACHE["nc"]


def _prep_inputs(selected_frames: np.ndarray, cluster_centers: np.ndarray):
    frames = np.ascontiguousarray(np.asarray(selected_frames, dtype=np.float32))
    centers = np.ascontiguousarray(np.asarray(cluster_centers, dtype=np.float32))

    # centers^T, D-major, chunked into 4 partition blocks of 128.
    cT = np.ascontiguousarray(centers.T.reshape(4, 128, N).astype(BF16))

    # -0.5*||c||^2: fp32 replicated (nottr) and bf16 hi/lo (packed).
    v = (-0.5 * (centers.astype(np.float64) ** 2).sum(-1)).astype(np.float32)
    csqb = np.ascontiguousarray(np.broadcast_to(v, (128, N)))
    hi = v.astype(BF16)
    lo = (v - hi.astype(np.float32)).astype(BF16)
    csq2 = np.ascontiguousarray(np.stack([hi, lo]))

    # block-ones map: column window [63-2t, 127-2t) is the [128, 64]
    # stationary matrix sending frame row f to batch row 2t + f//64.
    bmap = np.zeros((128, 127), dtype=BF16)
    bmap[0:64, 63] = 1
    bmap[64:128, 64] = 1

    in_maps = []
    fl = frames.reshape(B, K, D)
    for core in range(NCORES):
        fc = fl[core * B_PER_CORE:(core + 1) * B_PER_CORE].reshape(F_PER_CORE, D)
        fTc = np.ascontiguousarray(
            fc.reshape(NT, 128, 4, 128).transpose(3, 0, 2, 1)
        ).reshape(128, NT * 512).astype(BF16)
        fTc = np.ascontiguousarray(fTc)
        in_maps.append({"fT": fTc, "cT": cT, "csqb": csqb, "csq2": csq2,
                        "bmap": bmap})
    return in_maps


def _counts_to_cov(cnt: np.ndarray) -> np.ndarray:
    """Per-core count block -> coverage histogram [N]."""
    if cnt.shape[0] == 128:  # packed quadrant layout
        c0 = cnt[0:64, 0:512]
        c1 = cnt[64:128, 512:1024]
        cnt = np.concatenate([c0, c1], axis=1)
    return (cnt <= 63.5).sum(axis=0)


def kernel(selected_frames: np.ndarray, cluster_centers: np.ndarray) -> np.ndarray:
    global LAST_RESULTS
    nc = _get_nc()
    in_maps = _prep_inputs(selected_frames, cluster_centers)
    res = run_bass_kernel_spmd(nc, in_maps, list(range(NCORES)), **RUN_KWARGS)
    LAST_RESULTS = res
    cov = np.zeros(N, dtype=np.float64)
    for core in range(NCORES):
        cov += _counts_to_cov(res.results[core]["cnt"])
    prob = cov / (B * K)
    entropy = -(prob * np.log(prob + 1e-10)).sum()
    return np.float32(entropy)


if __name__ == "__main__":
    rng = np.random.default_rng(0)
    sf = rng.standard_normal((B, K, D), dtype=np.float32)
    cc = rng.standard_normal((N, D), dtype=np.float32)
    out = kernel(sf, cc)
    print("kernel out:", out)


# revision 37
# speedup vs baseline: 1.2904x; 1.0827x over previous
"""ClusterLoss (vq codebook coverage entropy) Trainium2 kernel.

Problem (hardcoded shapes):
    selected_frames [B=512, K=64, D=512] f32, cluster_centers [N=1024, D=512] f32.
    assignments = argmin_n ||x_{b,k} - c_n||^2
    coverage[n]  = sum_b 1[any_k assignments[b,k] == n]
    prob = coverage / (B*K);  out = -sum prob*log(prob+1e-10)   (scalar f32)

Strategy:
    * Data-parallel over batch: 64 batch elements (4096 frames) per core on 8
      cores; cluster_centers replicated.  Frames are pre-transposed on the
      host so every matmul operand is D-major, all in bf16 (measured effect:
      ~127/32768 flipped assignments, ~6.5e-4 relative entropy error).
    * argmin_n dist^2 == argmax_n s,  s = cross - 0.5*||c_n||^2.  Per
      128-frame tile, cross accumulates into fp32 PSUM via 8 bf16 matmuls at
      peak rate.
    * variant "packed" (default): the -0.5||c||^2 bias is added in-PSUM by a
      rank-2 bf16 (hi/lo) matmul pair; the two n-halves are placed in PE
      row-groups 0 and 1 (operands at base partitions 0 and 32) so they run
      concurrently in the array.  VectorE then does one PSUM row-max;
      ScalarE computes mask = sign(m - s) in {0 (argmax), 1}.
    * count[b, n] = sum_k mask via a block-ones stationary matmul pair,
      accumulated over all 32 tiles in persistent PSUM; the two n-halves go
      to PE column-groups (out partitions 0-63 / 64-127) and also run
      concurrently.
    * count PSUM -> SBUF -> DRAM; host: coverage[n] = sum_b 1[count<=63.5],
      then prob/entropy over the 8 per-core blocks.
    * variant "nottr": bias added on VectorE (tensor_tensor) instead of the
      rank-2 matmuls; row-max from SBUF.  Kept for A/B comparison.
"""

import os
import numpy as np
import ml_dtypes

import concourse.bass as bass
import concourse.mybir as mybir
import concourse.tile as tile
from concourse import bacc
from concourse.bass_utils import run_bass_kernel_spmd

BF16 = ml_dtypes.bfloat16
VARIANT = os.environ.get("KERNEL_VARIANT", "fused")


def _register_add_maxred():
    """Register a custom fused DVE op: out = in0 + in1 (full tensors),
    accum_out = rowmax(out).  One VectorE pass instead of
    tensor_tensor(add) + tensor_reduce(max)."""
    import concourse.dve_ops as dve_ops
    from concourse.dve_spec import Spec, Src0, Src1, AluOp, lower, _has_src1
    from concourse.dve_uop import DveOpSpec

    name = "TT_ADD_MAXRED_ANT"
    if name in dve_ops._SUB_OPCODE_FOR_NAME:
        return next(op for op in dve_ops.OPS if op.name == name)

    def _ref(in0, in1, s0, s1, imm2):
        b = in0.astype(np.float32) + in1.astype(np.float32)
        return b, b.reshape(b.shape[0], -1).max(axis=-1, keepdims=True)

    spec = Spec(body=Src0 + Src1, accum=AluOp.MAX, reference=_ref)
    row = max(dve_ops._SUB_OPCODE_FOR_NAME.values()) + 1
    assert row < 0x20
    # pin the sha of our own lowering (drift guard expects it declared)
    shas = {}
    for ver in ("v3",):
        tmp = DveOpSpec(name=name, opcode=row, uops=lower(spec, ver=ver),
                        rd1_en=_has_src1(spec))
        shas[ver] = tmp.sha(ver)
    op = dve_ops.DveOp(name, spec, subdim=False, uops_sha=shas)
    dve_ops._SUB_OPCODE_FOR_NAME[name] = row
    dve_ops.OPS.append(op)
    dve_ops.CUSTOM_DVE_SPECS[name] = spec
    return op

B, K, D, N = 512, 64, 512, 1024
NCORES = 8
B_PER_CORE = B // NCORES          # 64
F_PER_CORE = B_PER_CORE * K       # 4096 frames
NT = F_PER_CORE // 128            # 32 tiles of 128 frames (2 batch elems)
GROUP = 8                         # f-tiles per frame-load group
SUB = 2                           # DMAs per group
COUNT_DELAY = 3                   # tiles to delay the count matmul (pipelining)

_CACHE = {}
LAST_RESULTS = None
RUN_KWARGS = {}  # extra kwargs for run_bass_kernel_spmd (profiling harness hook)


def _build_nc():
    nc = bacc.Bacc("TRN2", target_bir_lowering=False, debug=False,
                   num_devices=NCORES)
    dt = mybir.dt
    packed = VARIANT == "packed"

    # DRAM I/O (per core).  fT layout: fT[d, t*512 + c*128 + f] =
    # frames[128*t + f, 128*c + d].
    fT = nc.dram_tensor("fT", [128, NT * 512], dt.bfloat16, kind="ExternalInput")
    cT = nc.dram_tensor("cT", [4, 128, N], dt.bfloat16, kind="ExternalInput")
    bmap = nc.dram_tensor("bmap", [128, 127], dt.bfloat16, kind="ExternalInput")
    cnt_out = nc.dram_tensor("cnt", [128, N], dt.float32, kind="ExternalOutput")
    if packed:
        csq2 = nc.dram_tensor("csq2", [2, N], dt.bfloat16, kind="ExternalInput")
    else:
        csqb = nc.dram_tensor("csqb", [128, N], dt.float32, kind="ExternalInput")

    with tile.TileContext(nc) as tc:
        with (
            tc.tile_pool(name="const", bufs=1) as cpool,
            tc.tile_pool(name="frames", bufs=2) as fpool,
            tc.tile_pool(name="sbias", bufs=3) as sbpool,
            tc.tile_pool(name="mask", bufs=COUNT_DELAY + 2) as mpool,
            tc.tile_pool(name="mrow", bufs=4) as mnpool,
            tc.tile_pool(name="scratch", bufs=2) as scpool,
            tc.tile_pool(name="spsum", bufs=2, space="PSUM") as spool,
            tc.tile_pool(name="cpsum", bufs=1, space="PSUM") as kpool,
            tc.tile_pool(name="outp", bufs=1) as opool,
        ):
            SUBW = GROUP * 512 // SUB
            fgs = {}

            def load_group(g):
                for u in range(SUB):
                    fg = fpool.tile([128, SUBW], dt.bfloat16, tag=f"fg{u}",
                                    name=f"fg{g}_{u}")
                    fgs[(g, u)] = fg
                    nc.gpsimd.dma_start(
                        out=fg[:],
                        in_=fT[:, g * GROUP * 512 + u * SUBW:
                               g * GROUP * 512 + (u + 1) * SUBW])

            # group 0 first half on SWDGE, then csqb (needed by the first ADD)
            # on the same queue, then the rest; centers stream on HWDGE.
            fg00 = fpool.tile([128, SUBW], dt.bfloat16, tag="fg0", name="fg0_0")
            fgs[(0, 0)] = fg00
            nc.gpsimd.dma_start(out=fg00[:], in_=fT[:, 0:SUBW])
            if not packed:
                csqb_t = cpool.tile([128, N], dt.float32, tag="csqb",
                                    name="csqb_t")
                nc.gpsimd.dma_start(out=csqb_t[:], in_=csqb[:])
            fg01 = fpool.tile([128, SUBW], dt.bfloat16, tag="fg1", name="fg0_1")
            fgs[(0, 1)] = fg01
            nc.gpsimd.dma_start(out=fg01[:], in_=fT[:, SUBW:2 * SUBW])

            ct = []
            for c in range(4):
                t_ = cpool.tile([128, N], dt.bfloat16, tag=f"ct{c}", name=f"ctt{c}")
                nc.sync.dma_start(out=t_[:], in_=cT[c])
                ct.append(t_)
            bmap_t = cpool.tile([128, 127], dt.bfloat16, tag="bmap", name="bmap_t")
            nc.sync.dma_start(out=bmap_t[:], in_=bmap[:])

            if packed:
                # bias operands at base partitions 0 (h=0) and 32 (h=1):
                # auto-derived tile_position puts the two rank-2 bias matmuls
                # in different PE row-groups so they run concurrently.
                csq2_t = cpool.tile([34, N], dt.bfloat16, tag="csq2",
                                    name="csq2_t")
                nc.sync.dma_start(out=csq2_t[0:2, :], in_=csq2[:])
                nc.sync.dma_start(out=csq2_t[32:34, :], in_=csq2[:])
                ones2_t = cpool.tile([34, 128], dt.bfloat16, tag="ones2",
                                     name="ones2_t")
                nc.vector.memset(ones2_t[0:2, :], 1.0)
                nc.vector.memset(ones2_t[32:34, :], 1.0)
                bias_ops = [(ones2_t[0:2, :], csq2_t[0:2, :]),
                            (ones2_t[32:34, :], csq2_t[32:34, :])]
            # persistent count accumulator: count = 64 - #argmax-hits.
            # Quadrant layout [128, 1024]: h=0 in rows 0:64 cols 0:512
            # (bank 0), h=1 in rows 64:128 cols 512:1024 (bank 1) — the two
            # count matmuls land in different PE column-groups and run
            # concurrently in the array.
            count = kpool.tile([128, N], dt.float32, tag="count", name="count")

            pending = []  # (t, mask_tile) with deferred count-matmul

            def flush_one():
                t, m = pending.pop(0)
                for h in range(2):
                    nc.tensor.matmul(
                        count[64 * h:64 * h + 64, h * 512:(h + 1) * 512],
                        lhsT=bmap_t[:, 63 - 2 * t:127 - 2 * t],
                        rhs=m[:, h * 512:(h + 1) * 512],
                        start=(t == 0), stop=(t == NT - 1),
                    )

            for g in range(NT // GROUP):
                if g + 1 < NT // GROUP:
                    load_group(g + 1)  # double-buffered prefetch
                for j in range(GROUP):
                    t = g * GROUP + j
                    fg = fgs[(g, j // (GROUP // SUB))]
                    jj = j % (GROUP // SUB)
                    s = spool.tile([128, N], dt.float32, tag="s", name=f"s{t}")
                    last_chunk = -1 if packed else 3
                    for c in range(4):
                        lhsT = fg[:, jj * 512 + c * 128: jj * 512 + (c + 1) * 128]
                        for h in range(2):
                            nc.tensor.matmul(
                                s[:, h * 512:(h + 1) * 512],
                                lhsT=lhsT,
                                rhs=ct[c][:, h * 512:(h + 1) * 512],
                                start=(c == 0), stop=(c == last_chunk),
                            )
                    mrow = mnpool.tile([128, 1], dt.float32, tag="mrow",
                                       name=f"mrow{t}")
                    if packed:
                        for h in range(2):
                            o2, cs2 = bias_ops[h]
                            nc.tensor.matmul(
                                s[:, h * 512:(h + 1) * 512],
                                lhsT=o2,
                                rhs=cs2[:, h * 512:(h + 1) * 512],
                                start=False, stop=True,
                            )
                        cmp_src = s
                        nc.vector.reduce_max(out=mrow[:], in_=s[:],
                                             axis=mybir.AxisListType.X)
                    else:
                        sb = sbpool.tile([128, N], dt.float32, tag="sb",
                                         name=f"sb{t}")
                        if VARIANT == "fused":
                            # one fused VectorE pass: sb = s + csqb and
                            # mrow = rowmax(sb) (custom DVE op)
                            nc.vector._custom_dve(
                                _register_add_maxred(), out=sb[:], in0=s[:],
                                in1=csqb_t[:], accum_out=mrow[:])
                        else:
                            nc.vector.tensor_tensor(
                                out=sb[:], in0=s[:], in1=csqb_t[:],
                                op=mybir.AluOpType.add)
                            nc.vector.reduce_max(out=mrow[:], in_=sb[:],
                                                 axis=mybir.AxisListType.X)
                        cmp_src = sb
                    # mask = sign(m - s) in {0 (argmax), 1 (other)}
                    mask = mpool.tile([128, N], dt.bfloat16, tag="mask",
                                      name=f"mask{t}")
                    nc.scalar.activation(mask[:], cmp_src[:],
                                         mybir.ActivationFunctionType.Sign,
                                         bias=mrow[:], scale=-1.0)
                    pending.append((t, mask))
                    if len(pending) > COUNT_DELAY:
                        flush_one()
            while pending:
                flush_one()

            res = opool.tile([128, N], dt.float32, tag="res", name="res")
            for h in range(2):
                sl = (slice(64 * h, 64 * h + 64), slice(h * 512, (h + 1) * 512))
                nc.scalar.copy(out=res[sl], in_=count[sl])
                nc.sync.dma_start(out=cnt_out[sl], in_=res[sl])

    nc.compile()
    return nc


def _get_nc():
    if "nc" not in _CACHE:
        _CACHE["nc"] = _build_nc()
    return _CACHE["nc"]


def _prep_inputs(selected_frames: np.ndarray, cluster_centers: np.ndarray):
    frames = np.ascontiguousarray(np.asarray(selected_frames, dtype=np.float32))
    centers = np.ascontiguousarray(np.asarray(cluster_centers, dtype=np.float32))

    # centers^T, D-major, chunked into 4 partition blocks of 128.
    cT = np.ascontiguousarray(centers.T.reshape(4, 128, N).astype(BF16))

    # -0.5*||c||^2: fp32 replicated (nottr) and bf16 hi/lo (packed).
    v = (-0.5 * (centers.astype(np.float64) ** 2).sum(-1)).astype(np.float32)
    csqb = np.ascontiguousarray(np.broadcast_to(v, (128, N)))
    hi = v.astype(BF16)
    lo = (v - hi.astype(np.float32)).astype(BF16)
    csq2 = np.ascontiguousarray(np.stack([hi, lo]))

    # block-ones map: column window [63-2t, 127-2t) is the [128, 64]
    # stationary matrix sending frame row f to batch row 2t + f//64.
    bmap = np.zeros((128, 127), dtype=BF16)
    bmap[0:64, 63] = 1
    bmap[64:128, 64] = 1

    in_maps = []
    fl = frames.reshape(B, K, D)
    for core in range(NCORES):
        fc = fl[core * B_PER_CORE:(core + 1) * B_PER_CORE].reshape(F_PER_CORE, D)
        fTc = np.ascontiguousarray(
            fc.reshape(NT, 128, 4, 128).transpose(3, 0, 2, 1)
        ).reshape(128, NT * 512).astype(BF16)
        fTc = np.ascontiguousarray(fTc)
        in_maps.append({"fT": fTc, "cT": cT, "csqb": csqb, "csq2": csq2,
                        "bmap": bmap})
    return in_maps


def _counts_to_cov(cnt: np.ndarray) -> np.ndarray:
    """Per-core count block -> coverage histogram [N]."""
    if cnt.shape[0] == 128:  # packed quadrant layout
        c0 = cnt[0:64, 0:512]
        c1 = cnt[64:128, 512:1024]
        cnt = np.concatenate([c0, c1], axis=1)
    return (cnt <= 63.5).sum(axis=0)


def kernel(selected_frames: np.ndarray, cluster_centers: np.ndarray) -> np.ndarray:
    global LAST_RESULTS
    nc = _get_nc()
    in_maps = _prep_inputs(selected_frames, cluster_centers)
    res = run_bass_kernel_spmd(nc, in_maps, list(range(NCORES)), **RUN_KWARGS)
    LAST_RESULTS = res
    cov = np.zeros(N, dtype=np.float64)
    for core in range(NCORES):
        cov += _counts_to_cov(res.results[core]["cnt"])
    prob = cov / (B * K)
    entropy = -(prob * np.log(prob + 1e-10)).sum()
    return np.float32(entropy)


if __name__ == "__main__":
    rng = np.random.default_rng(0)
    sf = rng.standard_normal((B, K, D), dtype=np.float32)
    cc = rng.standard_normal((N, D), dtype=np.float32)
    out = kernel(sf, cc)
    print("kernel out:", out)


# revision 41
# speedup vs baseline: 1.3074x; 1.0132x over previous
"""ClusterLoss (vq codebook coverage entropy) Trainium2 kernel.

Problem (hardcoded shapes):
    selected_frames [B=512, K=64, D=512] f32, cluster_centers [N=1024, D=512] f32.
    assignments = argmin_n ||x_{b,k} - c_n||^2
    coverage[n]  = sum_b 1[any_k assignments[b,k] == n]
    prob = coverage / (B*K);  out = -sum prob*log(prob+1e-10)   (scalar f32)

Strategy:
    * Data-parallel over batch: 64 batch elements (4096 frames) per core on 8
      cores; cluster_centers replicated.  Frames are pre-transposed on the
      host so every matmul operand is D-major, all in bf16 (measured effect:
      ~127/32768 flipped assignments, ~6.5e-4 relative entropy error).
    * argmin_n dist^2 == argmax_n s,  s = cross - 0.5*||c_n||^2.  Per
      128-frame tile, cross accumulates into fp32 PSUM via 8 bf16 matmuls at
      peak rate.
    * variant "packed" (default): the -0.5||c||^2 bias is added in-PSUM by a
      rank-2 bf16 (hi/lo) matmul pair; the two n-halves are placed in PE
      row-groups 0 and 1 (operands at base partitions 0 and 32) so they run
      concurrently in the array.  VectorE then does one PSUM row-max;
      ScalarE computes mask = sign(m - s) in {0 (argmax), 1}.
    * count[b, n] = sum_k mask via a block-ones stationary matmul pair,
      accumulated over all 32 tiles in persistent PSUM; the two n-halves go
      to PE column-groups (out partitions 0-63 / 64-127) and also run
      concurrently.
    * count PSUM -> SBUF -> DRAM; host: coverage[n] = sum_b 1[count<=63.5],
      then prob/entropy over the 8 per-core blocks.
    * variant "nottr": bias added on VectorE (tensor_tensor) instead of the
      rank-2 matmuls; row-max from SBUF.  Kept for A/B comparison.
"""

import os
import numpy as np
import ml_dtypes

import concourse.bass as bass
import concourse.mybir as mybir
import concourse.tile as tile
from concourse import bacc
from concourse.bass_utils import run_bass_kernel_spmd

BF16 = ml_dtypes.bfloat16
VARIANT = os.environ.get("KERNEL_VARIANT", "fused")


def _register_add_maxred():
    """Register a custom fused DVE op: out = in0 + in1 (full tensors),
    accum_out = rowmax(out).  One VectorE pass instead of
    tensor_tensor(add) + tensor_reduce(max)."""
    import concourse.dve_ops as dve_ops
    from concourse.dve_spec import Spec, Src0, Src1, AluOp, lower, _has_src1
    from concourse.dve_uop import DveOpSpec

    name = "TT_ADD_MAXRED_ANT"
    if name in dve_ops._SUB_OPCODE_FOR_NAME:
        return next(op for op in dve_ops.OPS if op.name == name)

    def _ref(in0, in1, s0, s1, imm2):
        b = in0.astype(np.float32) + in1.astype(np.float32)
        return b, b.reshape(b.shape[0], -1).max(axis=-1, keepdims=True)

    spec = Spec(body=Src0 + Src1, accum=AluOp.MAX, reference=_ref)
    row = max(dve_ops._SUB_OPCODE_FOR_NAME.values()) + 1
    assert row < 0x20
    # pin the sha of our own lowering (drift guard expects it declared)
    shas = {}
    for ver in ("v3",):
        tmp = DveOpSpec(name=name, opcode=row, uops=lower(spec, ver=ver),
                        rd1_en=_has_src1(spec))
        shas[ver] = tmp.sha(ver)
    op = dve_ops.DveOp(name, spec, subdim=False, uops_sha=shas)
    dve_ops._SUB_OPCODE_FOR_NAME[name] = row
    dve_ops.OPS.append(op)
    dve_ops.CUSTOM_DVE_SPECS[name] = spec
    return op

B, K, D, N = 512, 64, 512, 1024
NCORES = 8
B_PER_CORE = B // NCORES          # 64
F_PER_CORE = B_PER_CORE * K       # 4096 frames
NT = F_PER_CORE // 128            # 32 tiles of 128 frames (2 batch elems)
GROUP = 8                         # f-tiles per frame-load group
SUB = 2                           # DMAs per group
COUNT_DELAY = 3                   # tiles to delay the count matmul (pipelining)

_CACHE = {}
LAST_RESULTS = None
RUN_KWARGS = {}  # extra kwargs for run_bass_kernel_spmd (profiling harness hook)


def _build_nc():
    nc = bacc.Bacc("TRN2", target_bir_lowering=False, debug=False,
                   num_devices=NCORES)
    dt = mybir.dt
    packed = VARIANT == "packed"

    # DRAM I/O (per core).  fT layout: fT[d, t*512 + c*128 + f] =
    # frames[128*t + f, 128*c + d].
    fT = nc.dram_tensor("fT", [128, NT * 512], dt.bfloat16, kind="ExternalInput")
    cT = nc.dram_tensor("cT", [4, 128, N], dt.bfloat16, kind="ExternalInput")
    bmap = nc.dram_tensor("bmap", [128, 127], dt.bfloat16, kind="ExternalInput")
    cnt_out = nc.dram_tensor("cnt", [128, N], dt.float32, kind="ExternalOutput")
    if packed:
        csq2 = nc.dram_tensor("csq2", [2, N], dt.bfloat16, kind="ExternalInput")
    else:
        csqb = nc.dram_tensor("csqb", [128, N], dt.float32, kind="ExternalInput")

    with tile.TileContext(nc) as tc:
        with (
            tc.tile_pool(name="const", bufs=1) as cpool,
            tc.tile_pool(name="frames", bufs=2) as fpool,
            tc.tile_pool(name="sbias", bufs=3) as sbpool,
            tc.tile_pool(name="mask", bufs=COUNT_DELAY + 2) as mpool,
            tc.tile_pool(name="mrow", bufs=4) as mnpool,
            tc.tile_pool(name="scratch", bufs=2) as scpool,
            tc.tile_pool(name="spsum", bufs=2, space="PSUM") as spool,
            tc.tile_pool(name="cpsum", bufs=1, space="PSUM") as kpool,
            tc.tile_pool(name="outp", bufs=1) as opool,
        ):
            SUBW = GROUP * 512 // SUB
            fgs = {}

            def load_group(g):
                for u in range(SUB):
                    fg = fpool.tile([128, SUBW], dt.bfloat16, tag=f"fg{u}",
                                    name=f"fg{g}_{u}")
                    fgs[(g, u)] = fg
                    nc.gpsimd.dma_start(
                        out=fg[:],
                        in_=fT[:, g * GROUP * 512 + u * SUBW:
                               g * GROUP * 512 + (u + 1) * SUBW])

            # group 0 loads in small pieces so the first matmuls start early;
            # csqb (needed by the first fused add+max) right behind on the
            # same SWDGE queue; centers stream on HWDGE in parallel.
            fga = fpool.tile([128, 1024], dt.bfloat16, tag="fga", name="fga")
            nc.gpsimd.dma_start(out=fga[:], in_=fT[:, 0:1024])
            fgb = fpool.tile([128, 1024], dt.bfloat16, tag="fgb", name="fgb")
            nc.gpsimd.dma_start(out=fgb[:], in_=fT[:, 1024:2048])
            if not packed:
                csqb_t = cpool.tile([128, N], dt.float32, tag="csqb",
                                    name="csqb_t")
                nc.gpsimd.dma_start(out=csqb_t[:], in_=csqb[:])
            fg01 = fpool.tile([128, SUBW], dt.bfloat16, tag="fg1", name="fg0_1")
            fgs[(0, 1)] = fg01
            nc.gpsimd.dma_start(out=fg01[:], in_=fT[:, SUBW:2 * SUBW])

            def g0_tile(j):
                if j < 2:
                    return fga, j
                if j < 4:
                    return fgb, j - 2
                return fg01, j - 4

            ct = []
            for c in range(4):
                t_ = cpool.tile([128, N], dt.bfloat16, tag=f"ct{c}", name=f"ctt{c}")
                nc.sync.dma_start(out=t_[:], in_=cT[c])
                ct.append(t_)
            bmap_t = cpool.tile([128, 127], dt.bfloat16, tag="bmap", name="bmap_t")
            nc.sync.dma_start(out=bmap_t[:], in_=bmap[:])

            if packed:
                # bias operands at base partitions 0 (h=0) and 32 (h=1):
                # auto-derived tile_position puts the two rank-2 bias matmuls
                # in different PE row-groups so they run concurrently.
                csq2_t = cpool.tile([34, N], dt.bfloat16, tag="csq2",
                                    name="csq2_t")
                nc.sync.dma_start(out=csq2_t[0:2, :], in_=csq2[:])
                nc.sync.dma_start(out=csq2_t[32:34, :], in_=csq2[:])
                ones2_t = cpool.tile([34, 128], dt.bfloat16, tag="ones2",
                                     name="ones2_t")
                nc.vector.memset(ones2_t[0:2, :], 1.0)
                nc.vector.memset(ones2_t[32:34, :], 1.0)
                bias_ops = [(ones2_t[0:2, :], csq2_t[0:2, :]),
                            (ones2_t[32:34, :], csq2_t[32:34, :])]
            # persistent count accumulator: count = 64 - #argmax-hits.
            # Quadrant layout [128, 1024]: h=0 in rows 0:64 cols 0:512
            # (bank 0), h=1 in rows 64:128 cols 512:1024 (bank 1) — the two
            # count matmuls land in different PE column-groups and run
            # concurrently in the array.
            count = kpool.tile([128, N], dt.float32, tag="count", name="count")

            pending = []  # (t, mask_tile) with deferred count-matmul

            def flush_one():
                t, m = pending.pop(0)
                for h in range(2):
                    nc.tensor.matmul(
                        count[64 * h:64 * h + 64, h * 512:(h + 1) * 512],
                        lhsT=bmap_t[:, 63 - 2 * t:127 - 2 * t],
                        rhs=m[:, h * 512:(h + 1) * 512],
                        start=(t == 0), stop=(t == NT - 1),
                    )

            for g in range(NT // GROUP):
                if g + 1 < NT // GROUP:
                    load_group(g + 1)  # double-buffered prefetch
                for j in range(GROUP):
                    t = g * GROUP + j
                    if g == 0:
                        fg, jj = g0_tile(j)
                    else:
                        fg = fgs[(g, j // (GROUP // SUB))]
                        jj = j % (GROUP // SUB)
                    s = spool.tile([128, N], dt.float32, tag="s", name=f"s{t}")
                    last_chunk = -1 if packed else 3
                    for c in range(4):
                        lhsT = fg[:, jj * 512 + c * 128: jj * 512 + (c + 1) * 128]
                        for h in range(2):
                            nc.tensor.matmul(
                                s[:, h * 512:(h + 1) * 512],
                                lhsT=lhsT,
                                rhs=ct[c][:, h * 512:(h + 1) * 512],
                                start=(c == 0), stop=(c == last_chunk),
                            )
                    mrow = mnpool.tile([128, 1], dt.float32, tag="mrow",
                                       name=f"mrow{t}")
                    if packed:
                        for h in range(2):
                            o2, cs2 = bias_ops[h]
                            nc.tensor.matmul(
                                s[:, h * 512:(h + 1) * 512],
                                lhsT=o2,
                                rhs=cs2[:, h * 512:(h + 1) * 512],
                                start=False, stop=True,
                            )
                        cmp_src = s
                        nc.vector.reduce_max(out=mrow[:], in_=s[:],
                                             axis=mybir.AxisListType.X)
                    else:
                        sb = sbpool.tile([128, N], dt.float32, tag="sb",
                                         name=f"sb{t}")
                        if VARIANT == "fused":
                            # one fused VectorE pass: sb = s + csqb and
                            # mrow = rowmax(sb) (custom DVE op)
                            nc.vector._custom_dve(
                                _register_add_maxred(), out=sb[:], in0=s[:],
                                in1=csqb_t[:], accum_out=mrow[:])
                        else:
                            nc.vector.tensor_tensor(
                                out=sb[:], in0=s[:], in1=csqb_t[:],
                                op=mybir.AluOpType.add)
                            nc.vector.reduce_max(out=mrow[:], in_=sb[:],
                                                 axis=mybir.AxisListType.X)
                        cmp_src = sb
                    # mask = sign(m - s) in {0 (argmax), 1 (other)}
                    mask = mpool.tile([128, N], dt.bfloat16, tag="mask",
                                      name=f"mask{t}")
                    if t == NT - 1:
                        # split the last sign so the final count matmuls can
                        # start after the first half (shorter kernel tail)
                        for h in range(2):
                            nc.scalar.activation(
                                mask[:, h * 512:(h + 1) * 512],
                                cmp_src[:, h * 512:(h + 1) * 512],
                                mybir.ActivationFunctionType.Sign,
                                bias=mrow[:], scale=-1.0)
                    else:
                        nc.scalar.activation(mask[:], cmp_src[:],
                                             mybir.ActivationFunctionType.Sign,
                                             bias=mrow[:], scale=-1.0)
                    pending.append((t, mask))
                    if len(pending) > COUNT_DELAY:
                        flush_one()
            while pending:
                flush_one()

            res = opool.tile([128, N], dt.float32, tag="res", name="res")
            sl0 = (slice(0, 64), slice(0, 512))
            sl1 = (slice(64, 128), slice(512, 1024))
            nc.scalar.copy(out=res[sl0], in_=count[sl0])
            nc.vector.tensor_copy(res[sl1], count[sl1])  # parallel on DVE
            nc.sync.dma_start(out=cnt_out[sl0], in_=res[sl0])
            nc.sync.dma_start(out=cnt_out[sl1], in_=res[sl1])

    nc.compile()
    return nc


def _get_nc():
    if "nc" not in _CACHE:
        _CACHE["nc"] = _build_nc()
    return _CACHE["nc"]


def _prep_inputs(selected_frames: np.ndarray, cluster_centers: np.ndarray):
    frames = np.ascontiguousarray(np.asarray(selected_frames, dtype=np.float32))
    centers = np.ascontiguousarray(np.asarray(cluster_centers, dtype=np.float32))

    # centers^T, D-major, chunked into 4 partition blocks of 128.
    cT = np.ascontiguousarray(centers.T.reshape(4, 128, N).astype(BF16))

    # -0.5*||c||^2: fp32 replicated (nottr) and bf16 hi/lo (packed).
    v = (-0.5 * (centers.astype(np.float64) ** 2).sum(-1)).astype(np.float32)
    csqb = np.ascontiguousarray(np.broadcast_to(v, (128, N)))
    hi = v.astype(BF16)
    lo = (v - hi.astype(np.float32)).astype(BF16)
    csq2 = np.ascontiguousarray(np.stack([hi, lo]))

    # block-ones map: column window [63-2t, 127-2t) is the [128, 64]
    # stationary matrix sending frame row f to batch row 2t + f//64.
    bmap = np.zeros((128, 127), dtype=BF16)
    bmap[0:64, 63] = 1
    bmap[64:128, 64] = 1

    in_maps = []
    fl = frames.reshape(B, K, D)
    for core in range(NCORES):
        fc = fl[core * B_PER_CORE:(core + 1) * B_PER_CORE].reshape(F_PER_CORE, D)
        fTc = np.ascontiguousarray(
            fc.reshape(NT, 128, 4, 128).transpose(3, 0, 2, 1)
        ).reshape(128, NT * 512).astype(BF16)
        fTc = np.ascontiguousarray(fTc)
        in_maps.append({"fT": fTc, "cT": cT, "csqb": csqb, "csq2": csq2,
                        "bmap": bmap})
    return in_maps


def _counts_to_cov(cnt: np.ndarray) -> np.ndarray:
    """Per-core count block -> coverage histogram [N]."""
    if cnt.shape[0] == 128:  # packed quadrant layout
        c0 = cnt[0:64, 0:512]
        c1 = cnt[64:128, 512:1024]
        cnt = np.concatenate([c0, c1], axis=1)
    return (cnt <= 63.5).sum(axis=0)


def kernel(selected_frames: np.ndarray, cluster_centers: np.ndarray) -> np.ndarray:
    global LAST_RESULTS
    nc = _get_nc()
    in_maps = _prep_inputs(selected_frames, cluster_centers)
    res = run_bass_kernel_spmd(nc, in_maps, list(range(NCORES)), **RUN_KWARGS)
    LAST_RESULTS = res
    cov = np.zeros(N, dtype=np.float64)
    for core in range(NCORES):
        cov += _counts_to_cov(res.results[core]["cnt"])
    prob = cov / (B * K)
    entropy = -(prob * np.log(prob + 1e-10)).sum()
    return np.float32(entropy)


if __name__ == "__main__":
    rng = np.random.default_rng(0)
    sf = rng.standard_normal((B, K, D), dtype=np.float32)
    cc = rng.standard_normal((N, D), dtype=np.float32)
    out = kernel(sf, cc)
    print("kernel out:", out)


# revision 49
# speedup vs baseline: 1.3479x; 1.0310x over previous
"""ClusterLoss (vq codebook coverage entropy) Trainium2 kernel.

Problem (hardcoded shapes):
    selected_frames [B=512, K=64, D=512] f32, cluster_centers [N=1024, D=512] f32.
    assignments = argmin_n ||x_{b,k} - c_n||^2
    coverage[n]  = sum_b 1[any_k assignments[b,k] == n]
    prob = coverage / (B*K);  out = -sum prob*log(prob+1e-10)   (scalar f32)

Strategy:
    * Data-parallel over batch: 64 batch elements (4096 frames) per core on 8
      cores; cluster_centers replicated.  Frames are pre-transposed on the
      host so every matmul operand is D-major, all in bf16 (measured effect:
      ~127/32768 flipped assignments, ~6.5e-4 relative entropy error).
    * argmin_n dist^2 == argmax_n s,  s = cross - 0.5*||c_n||^2.  Per
      128-frame tile, cross accumulates into fp32 PSUM via 8 bf16 matmuls at
      peak rate.
    * variant "packed" (default): the -0.5||c||^2 bias is added in-PSUM by a
      rank-2 bf16 (hi/lo) matmul pair; the two n-halves are placed in PE
      row-groups 0 and 1 (operands at base partitions 0 and 32) so they run
      concurrently in the array.  VectorE then does one PSUM row-max;
      ScalarE computes mask = sign(m - s) in {0 (argmax), 1}.
    * count[b, n] = sum_k mask via a block-ones stationary matmul pair,
      accumulated over all 32 tiles in persistent PSUM; the two n-halves go
      to PE column-groups (out partitions 0-63 / 64-127) and also run
      concurrently.
    * count PSUM -> SBUF -> DRAM; host: coverage[n] = sum_b 1[count<=63.5],
      then prob/entropy over the 8 per-core blocks.
    * variant "nottr": bias added on VectorE (tensor_tensor) instead of the
      rank-2 matmuls; row-max from SBUF.  Kept for A/B comparison.
"""

import os
import numpy as np
import ml_dtypes

import concourse.bass as bass
import concourse.mybir as mybir
import concourse.tile as tile
from concourse import bacc
from concourse.bass_utils import run_bass_kernel_spmd

BF16 = ml_dtypes.bfloat16
VARIANT = os.environ.get("KERNEL_VARIANT", "fused")


def _register_add_maxred():
    """Register a custom fused DVE op: out = in0 + in1 (full tensors),
    accum_out = rowmax(out).  One VectorE pass instead of
    tensor_tensor(add) + tensor_reduce(max)."""
    import concourse.dve_ops as dve_ops
    from concourse.dve_spec import Spec, Src0, Src1, AluOp, lower, _has_src1
    from concourse.dve_uop import DveOpSpec

    name = "TT_ADD_MAXRED_ANT"
    if name in dve_ops._SUB_OPCODE_FOR_NAME:
        return next(op for op in dve_ops.OPS if op.name == name)

    def _ref(in0, in1, s0, s1, imm2):
        b = in0.astype(np.float32) + in1.astype(np.float32)
        return b, b.reshape(b.shape[0], -1).max(axis=-1, keepdims=True)

    spec = Spec(body=Src0 + Src1, accum=AluOp.MAX, reference=_ref)
    row = max(dve_ops._SUB_OPCODE_FOR_NAME.values()) + 1
    assert row < 0x20
    # pin the sha of our own lowering (drift guard expects it declared)
    shas = {}
    for ver in ("v3",):
        tmp = DveOpSpec(name=name, opcode=row, uops=lower(spec, ver=ver),
                        rd1_en=_has_src1(spec))
        shas[ver] = tmp.sha(ver)
    op = dve_ops.DveOp(name, spec, subdim=False, uops_sha=shas)
    dve_ops._SUB_OPCODE_FOR_NAME[name] = row
    dve_ops.OPS.append(op)
    dve_ops.CUSTOM_DVE_SPECS[name] = spec
    return op

B, K, D, N = 512, 64, 512, 1024
NCORES = 8
B_PER_CORE = B // NCORES          # 64
F_PER_CORE = B_PER_CORE * K       # 4096 frames
NT = F_PER_CORE // 128            # 32 tiles of 128 frames (2 batch elems)
GROUP = 8                         # f-tiles per frame-load group
SUB = 2                           # DMAs per group
COUNT_DELAY = 3                   # tiles to delay the count matmul (pipelining)

_CACHE = {}
LAST_RESULTS = None
RUN_KWARGS = {}  # extra kwargs for run_bass_kernel_spmd (profiling harness hook)


def _build_nc():
    nc = bacc.Bacc("TRN2", target_bir_lowering=False, debug=False,
                   num_devices=NCORES)
    dt = mybir.dt
    packed = VARIANT == "packed"

    # DRAM I/O (per core).  fT layout: fT[d, t*512 + c*128 + f] =
    # frames[128*t + f, 128*c + d].
    fT = nc.dram_tensor("fT", [128, NT * 512], dt.bfloat16, kind="ExternalInput")
    cT = nc.dram_tensor("cT", [4, 128, N], dt.bfloat16, kind="ExternalInput")
    bmap = nc.dram_tensor("bmap", [128, 127], dt.bfloat16, kind="ExternalInput")
    cnt_out = nc.dram_tensor("cnt", [128, N], dt.float32, kind="ExternalOutput")
    if packed:
        csq2 = nc.dram_tensor("csq2", [2, N], dt.bfloat16, kind="ExternalInput")
    else:
        csqb = nc.dram_tensor("csqb", [128, N], dt.float32, kind="ExternalInput")
    DR = VARIANT == "dr"
    if DR:
        bmap2 = nc.dram_tensor("bmap2", [128, 256], dt.float8e4,
                               kind="ExternalInput")

    with tile.TileContext(nc) as tc:
        with (
            tc.tile_pool(name="const", bufs=1) as cpool,
            tc.tile_pool(name="frames", bufs=2) as fpool,
            tc.tile_pool(name="sbias", bufs=3) as sbpool,
            tc.tile_pool(name="mask", bufs=COUNT_DELAY + 2) as mpool,
            tc.tile_pool(name="mrow", bufs=4) as mnpool,
            tc.tile_pool(name="scratch", bufs=2) as scpool,
            tc.tile_pool(name="spsum", bufs=2, space="PSUM") as spool,
            tc.tile_pool(name="cpsum", bufs=1, space="PSUM") as kpool,
            tc.tile_pool(name="outp", bufs=1) as opool,
        ):
            SUBW = GROUP * 512 // SUB
            fgs = {}

            def load_group(g):
                for u in range(SUB):
                    fg = fpool.tile([128, SUBW], dt.bfloat16, tag=f"fg{u}",
                                    name=f"fg{g}_{u}")
                    fgs[(g, u)] = fg
                    nc.gpsimd.dma_start(
                        out=fg[:],
                        in_=fT[:, g * GROUP * 512 + u * SUBW:
                               g * GROUP * 512 + (u + 1) * SUBW])

            # group 0 loads in small pieces so the first matmuls start early;
            # csqb (needed by the first fused add+max) right behind on the
            # same SWDGE queue; centers stream on HWDGE in parallel.
            fga = fpool.tile([128, 1024], dt.bfloat16, tag="fga", name="fga")
            nc.gpsimd.dma_start(out=fga[:], in_=fT[:, 0:1024])
            fgb = fpool.tile([128, 1024], dt.bfloat16, tag="fgb", name="fgb")
            nc.gpsimd.dma_start(out=fgb[:], in_=fT[:, 1024:2048])
            if not packed:
                csqb_t = cpool.tile([128, N], dt.float32, tag="csqb",
                                    name="csqb_t")
                nc.gpsimd.dma_start(out=csqb_t[:], in_=csqb[:])
            fg01 = fpool.tile([128, SUBW], dt.bfloat16, tag="fg1", name="fg0_1")
            fgs[(0, 1)] = fg01
            nc.gpsimd.dma_start(out=fg01[:], in_=fT[:, SUBW:2 * SUBW])

            def g0_tile(j):
                if j < 2:
                    return fga, j
                if j < 4:
                    return fgb, j - 2
                return fg01, j - 4

            ct = []
            for c in range(4):
                t_ = cpool.tile([128, N], dt.bfloat16, tag=f"ct{c}", name=f"ctt{c}")
                nc.sync.dma_start(out=t_[:], in_=cT[c])
                ct.append(t_)
            bmap_t = cpool.tile([128, 127], dt.bfloat16, tag="bmap", name="bmap_t")
            nc.sync.dma_start(out=bmap_t[:], in_=bmap[:])
            if DR:
                bmap2_t = cpool.tile([128, 256], dt.float8e4, tag="bmap2",
                                     name="bmap2_t")
                nc.sync.dma_start(out=bmap2_t[:], in_=bmap2[:])
                bmap2_v = bmap2_t[:].rearrange("p (two c) -> p two c", two=2)

            if packed:
                # bias operands at base partitions 0 (h=0) and 32 (h=1):
                # auto-derived tile_position puts the two rank-2 bias matmuls
                # in different PE row-groups so they run concurrently.
                csq2_t = cpool.tile([34, N], dt.bfloat16, tag="csq2",
                                    name="csq2_t")
                nc.sync.dma_start(out=csq2_t[0:2, :], in_=csq2[:])
                nc.sync.dma_start(out=csq2_t[32:34, :], in_=csq2[:])
                ones2_t = cpool.tile([34, 128], dt.bfloat16, tag="ones2",
                                     name="ones2_t")
                nc.vector.memset(ones2_t[0:2, :], 1.0)
                nc.vector.memset(ones2_t[32:34, :], 1.0)
                bias_ops = [(ones2_t[0:2, :], csq2_t[0:2, :]),
                            (ones2_t[32:34, :], csq2_t[32:34, :])]
            # persistent count accumulator: count = 64 - #argmax-hits.
            # Quadrant layout [128, 1024]: h=0 in rows 0:64 cols 0:512
            # (bank 0), h=1 in rows 64:128 cols 512:1024 (bank 1) — the two
            # count matmuls land in different PE column-groups and run
            # concurrently in the array.
            count = kpool.tile([128, N], dt.float32, tag="count", name="count")

            pending = []  # (t-or-pair, mask_tile) with deferred count-matmul

            def flush_one():
                t, m = pending.pop(0)
                if DR:
                    p = t  # pair index; m is the [128, 2048] fp8 pair tile
                    mv = m.rearrange("q (two n) -> q two n", two=2)
                    for h in range(2):
                        nc.tensor.matmul(
                            count[64 * h:64 * h + 64, h * 512:(h + 1) * 512],
                            lhsT=bmap2_v[:, :, 63 - 4 * p:127 - 4 * p],
                            rhs=mv[:, :, h * 512:(h + 1) * 512],
                            start=(p == 0), stop=(p == NT // 2 - 1),
                            perf_mode=mybir.MatmulPerfMode.DoubleRow,
                        )
                    return
                for h in range(2):
                    nc.tensor.matmul(
                        count[64 * h:64 * h + 64, h * 512:(h + 1) * 512],
                        lhsT=bmap_t[:, 63 - 2 * t:127 - 2 * t],
                        rhs=m[:, h * 512:(h + 1) * 512],
                        start=(t == 0), stop=(t == NT - 1),
                    )

            holder = {}
            for g in range(NT // GROUP):
                if g + 1 < NT // GROUP:
                    load_group(g + 1)  # double-buffered prefetch
                for j in range(GROUP):
                    t = g * GROUP + j
                    if g == 0:
                        fg, jj = g0_tile(j)
                    else:
                        fg = fgs[(g, j // (GROUP // SUB))]
                        jj = j % (GROUP // SUB)
                    s = spool.tile([128, N], dt.float32, tag="s", name=f"s{t}")
                    last_chunk = -1 if packed else 3
                    for c in range(4):
                        lhsT = fg[:, jj * 512 + c * 128: jj * 512 + (c + 1) * 128]
                        for h in range(2):
                            nc.tensor.matmul(
                                s[:, h * 512:(h + 1) * 512],
                                lhsT=lhsT,
                                rhs=ct[c][:, h * 512:(h + 1) * 512],
                                start=(c == 0), stop=(c == last_chunk),
                            )
                    mrow = mnpool.tile([128, 1], dt.float32, tag="mrow",
                                       name=f"mrow{t}")
                    if packed:
                        for h in range(2):
                            o2, cs2 = bias_ops[h]
                            nc.tensor.matmul(
                                s[:, h * 512:(h + 1) * 512],
                                lhsT=o2,
                                rhs=cs2[:, h * 512:(h + 1) * 512],
                                start=False, stop=True,
                            )
                        cmp_src = s
                        nc.vector.reduce_max(out=mrow[:], in_=s[:],
                                             axis=mybir.AxisListType.X)
                    else:
                        sb = sbpool.tile([128, N], dt.float32, tag="sb",
                                         name=f"sb{t}")
                        if VARIANT == "fused":
                            # one fused VectorE pass: sb = s + csqb and
                            # mrow = rowmax(sb) (custom DVE op)
                            nc.vector._custom_dve(
                                _register_add_maxred(), out=sb[:], in0=s[:],
                                in1=csqb_t[:], accum_out=mrow[:])
                        else:
                            nc.vector.tensor_tensor(
                                out=sb[:], in0=s[:], in1=csqb_t[:],
                                op=mybir.AluOpType.add)
                            nc.vector.reduce_max(out=mrow[:], in_=sb[:],
                                                 axis=mybir.AxisListType.X)
                        cmp_src = sb
                    # mask = sign(m - s) in {0 (argmax), 1 (other)}
                    if DR:
                        if t % 2 == 0:
                            holder["pair"] = mpool.tile(
                                [128, 2 * N], dt.float8e4, tag="mask",
                                name=f"maskp{t // 2}")
                        mask = holder["pair"][:, (t % 2) * N:(t % 2) * N + N]
                    else:
                        mask = mpool.tile([128, N], dt.bfloat16, tag="mask",
                                          name=f"mask{t}")[:]
                    if t == NT - 1:
                        # split the last sign so the final count matmuls can
                        # start after the first half (shorter kernel tail)
                        for h in range(2):
                            nc.scalar.activation(
                                mask[:, h * 512:(h + 1) * 512],
                                cmp_src[:, h * 512:(h + 1) * 512],
                                mybir.ActivationFunctionType.Sign,
                                bias=mrow[:], scale=-1.0)
                    else:
                        nc.scalar.activation(mask[:], cmp_src[:],
                                             mybir.ActivationFunctionType.Sign,
                                             bias=mrow[:], scale=-1.0)
                    if DR:
                        if t % 2 == 1:
                            pending.append((t // 2, holder["pair"]))
                            if len(pending) > 2:
                                flush_one()
                    else:
                        pending.append((t, mask))
                        if len(pending) > COUNT_DELAY:
                            flush_one()
            while pending:
                flush_one()

            res = opool.tile([128, N], dt.float32, tag="res", name="res")
            sl0 = (slice(0, 64), slice(0, 512))
            sl1 = (slice(64, 128), slice(512, 1024))
            nc.scalar.copy(out=res[sl0], in_=count[sl0])
            nc.vector.tensor_copy(res[sl1], count[sl1])  # parallel on DVE
            nc.sync.dma_start(out=cnt_out[sl0], in_=res[sl0])
            nc.sync.dma_start(out=cnt_out[sl1], in_=res[sl1])

    nc.compile()
    return nc


def _get_nc():
    if "nc" not in _CACHE:
        _CACHE["nc"] = _build_nc()
    return _CACHE["nc"]


def _prep_inputs(selected_frames: np.ndarray, cluster_centers: np.ndarray):
    frames = np.ascontiguousarray(np.asarray(selected_frames, dtype=np.float32))
    centers = np.ascontiguousarray(np.asarray(cluster_centers, dtype=np.float32))

    # centers^T, D-major, chunked into 4 partition blocks of 128.
    cT = np.ascontiguousarray(centers.T.reshape(4, 128, N).astype(BF16))

    # -0.5*||c||^2: fp32 replicated (nottr) and bf16 hi/lo (packed).
    v = (-0.5 * (centers.astype(np.float64) ** 2).sum(-1)).astype(np.float32)
    csqb = np.ascontiguousarray(np.broadcast_to(v, (128, N)))
    hi = v.astype(BF16)
    lo = (v - hi.astype(np.float32)).astype(BF16)
    csq2 = np.ascontiguousarray(np.stack([hi, lo]))

    # block-ones map: column window [63-2t, 127-2t) is the [128, 64]
    # stationary matrix sending frame row f to batch row 2t + f//64.
    bmap = np.zeros((128, 127), dtype=BF16)
    bmap[0:64, 63] = 1
    bmap[64:128, 64] = 1
    # DoubleRow variant: two plane windows in one [128, 256] fp8 constant
    bmap2 = np.zeros((128, 256), dtype=ml_dtypes.float8_e4m3)
    bmap2[0:64, 63] = 1
    bmap2[64:128, 64] = 1
    bmap2[0:64, 193] = 1
    bmap2[64:128, 194] = 1

    in_maps = []
    fl = frames.reshape(B, K, D)
    for core in range(NCORES):
        fc = fl[core * B_PER_CORE:(core + 1) * B_PER_CORE].reshape(F_PER_CORE, D)
        fTc = np.ascontiguousarray(
            fc.reshape(NT, 128, 4, 128).transpose(3, 0, 2, 1)
        ).reshape(128, NT * 512).astype(BF16)
        fTc = np.ascontiguousarray(fTc)
        in_maps.append({"fT": fTc, "cT": cT, "csqb": csqb, "csq2": csq2,
                        "bmap": bmap, "bmap2": bmap2})
    return in_maps


def _counts_to_cov(cnt: np.ndarray) -> np.ndarray:
    """Per-core count block -> coverage histogram [N]."""
    if cnt.shape[0] == 128:  # packed quadrant layout
        c0 = cnt[0:64, 0:512]
        c1 = cnt[64:128, 512:1024]
        cnt = np.concatenate([c0, c1], axis=1)
    return (cnt <= 63.5).sum(axis=0)


def kernel(selected_frames: np.ndarray, cluster_centers: np.ndarray) -> np.ndarray:
    global LAST_RESULTS
    nc = _get_nc()
    in_maps = _prep_inputs(selected_frames, cluster_centers)
    res = run_bass_kernel_spmd(nc, in_maps, list(range(NCORES)), **RUN_KWARGS)
    LAST_RESULTS = res
    cov = np.zeros(N, dtype=np.float64)
    for core in range(NCORES):
        cov += _counts_to_cov(res.results[core]["cnt"])
    prob = cov / (B * K)
    entropy = -(prob * np.log(prob + 1e-10)).sum()
    return np.float32(entropy)


if __name__ == "__main__":
    rng = np.random.default_rng(0)
    sf = rng.standard_normal((B, K, D), dtype=np.float32)
    cc = rng.standard_normal((N, D), dtype=np.float32)
    out = kernel(sf, cc)
    print("kernel out:", out)
